# revision 6
# baseline (speedup 1.0000x reference)
"""AttentionAugmentedConv2d Trainium2 kernel (8 NeuronCores, SPMD).

Sharding: core c -> image b = c//2, half g = c%2.
Each core computes, for its image:
  - conv3x3 for 128 of the 256 conv_out channels
  - qkv conv3x3 for its 4 heads (128 q + 128 k + 128 v channels)
  - full attention (with relative position logits) for those 4 heads
  - a partial 1x1 "combine" conv: all 256 output channels contracted over
    its 128 att channels.  Host sums the two halves per image (att bias is
    folded into the g==0 core's bias input).

All matmuls run in float32r (tf32-like, ~2e-4 rel err) at full PE rate.
Relative logits are folded into the QK matmul by augmenting the
contraction dim: logits^T = [k; Ex; Ey]^T @ [q; RX^T; RY^T], with Ex/Ey
constant 0/1 indicator matrices and RX/RY produced by a small q @ kr^T
matmul followed by a shear-gather DMA through DRAM (the rel_to_abs skew
is a linear access pattern in (y, x, c) coordinates).

Logits are computed transposed (keys on partitions, queries free) so that
softmax is exp (no max subtraction; a constant -20 bias guards overflow
and cancels in normalization) and the AV matmul needs no 1024x1024
transpose; denominators come from a ones-column appended to fv^T.
The reference's raw reshape of att (B,NH,HW,dvh)->(B,NH,dvh,H,W) is a
flat memory reinterpretation, obtained for free via a DRAM roundtrip.
"""

import numpy as np

NH = 8
DK = 256
DV = 256
DKH = DK // NH          # 32
DVH = DV // NH          # 32
H = W = 32
HW = H * W              # 1024
B = 4
CIN = 128
COUT = 512
CONV_CH = COUT - DV     # 256
KS = 3
NCORES = 8
HPC = NH // 2           # heads per core = 4
PADW = W + 2            # 34
NPIX = PADW * PADW      # 1156
RELW = 2 * W - 1        # 63
RELC = 2 * RELW         # 126

_CACHE = {}


def _build_bass():
    import concourse.bass as bass
    import concourse.mybir as mybir
    from concourse import bacc
    from concourse.tile import TileContext
    from concourse.masks import make_identity

    dt = mybir.dt
    f32 = dt.float32
    f32r = dt.float32r
    AF = mybir.ActivationFunctionType

    nc = bacc.Bacc("TRN2", target_bir_lowering=False, debug=False,
                   num_devices=NCORES)

    # ---- I/O ----
    xp_d = nc.dram_tensor("xp", [CIN, NPIX], f32r, kind="ExternalInput")
    wall_d = nc.dram_tensor("wall", [CIN, 4 * 9 * 128], f32r, kind="ExternalInput")
    ball_d = nc.dram_tensor("ball", [4, 128], f32, kind="ExternalInput")
    krbd_d = nc.dram_tensor("krbd", [128, HPC * RELC], f32r, kind="ExternalInput")
    exey_d = nc.dram_tensor("exey", [64, HW], f32r, kind="ExternalInput")
    awt_d = nc.dram_tensor("awt", [128, 256], f32r, kind="ExternalInput")
    ab_d = nc.dram_tensor("ab", [2, 128], f32, kind="ExternalInput")
    oc_d = nc.dram_tensor("oc", [128, HW], f32, kind="ExternalOutput")
    oa_d = nc.dram_tensor("oa", [256, HW], f32, kind="ExternalOutput")
    # internal DRAM scratch
    prod_d = nc.dram_tensor("prodd", [HW, HPC * RELC], f32r)
    att_d = nc.dram_tensor("attd", [128, HW], f32r)

    PRODW = HPC * RELC  # 504

    with TileContext(nc) as tc:
        with (
            tc.tile_pool(name="consts", bufs=1) as consts,
            tc.tile_pool(name="qkv", bufs=1) as qkvp,
            tc.tile_pool(name="haug", bufs=2) as haug,
            tc.tile_pool(name="expp", bufs=3) as expp,
            tc.tile_pool(name="work", bufs=3) as work,
            tc.tile_pool(name="outp", bufs=2) as outp,
            tc.tile_pool(name="psA", bufs=2, space="PSUM") as psA,
            tc.tile_pool(name="psST", bufs=2, space="PSUM") as psST,
            tc.tile_pool(name="psAV", bufs=1, space="PSUM") as psAV,
            tc.tile_pool(name="psT", bufs=2, space="PSUM") as psT,
        ):
            # ---- constants / inputs to SBUF ----
            xp_sb = consts.tile([CIN, NPIX], f32r)
            nc.sync.dma_start(xp_sb[:], xp_d[:])
            wall_sb = consts.tile([CIN, 4, 9, 128], f32r)
            nc.sync.dma_start(wall_sb[:], wall_d[:].rearrange(
                "c (g t o) -> c g t o", g=4, t=9))
            ball_sb = consts.tile([128, 4], f32)
            nc.sync.dma_start(ball_sb[:], ball_d[:].rearrange("g c -> c g"))
            krbd_sb = consts.tile([128, PRODW], f32r)
            nc.sync.dma_start(krbd_sb[:], krbd_d[:])
            exey_sb = consts.tile([64, HW], f32r)
            nc.sync.dma_start(exey_sb[:], exey_d[:])
            awt_sb = consts.tile([128, 256], f32r)
            nc.sync.dma_start(awt_sb[:], awt_d[:])
            ab_sb = consts.tile([128, 2], f32)
            nc.sync.dma_start(ab_sb[:], ab_d[:].rearrange("g c -> c g"))

            ident = consts.tile([128, 128], f32)
            make_identity(nc, ident[:])
            ones_f = consts.tile([128, 1], f32)
            nc.gpsimd.memset(ones_f[:], 1.0)
            ones_r = consts.tile([128, 1], f32r)
            nc.vector.tensor_copy(ones_r[:], ones_f[:])
            negc = consts.tile([128, 1], f32)
            nc.gpsimd.memset(negc[:], -20.0)

            # ---- convs: grp 0 = conv_out half, 1/2/3 = q/k/v ----
            q_all = qkvp.tile([128, HW], f32r, tag="q")
            k_all = qkvp.tile([128, HW], f32r, tag="k")
            v_all = qkvp.tile([128, HW], f32r, tag="v")
            xp3 = xp_sb[:].rearrange("c (a b) -> c a b", a=PADW)
            qkv_dst = [None, q_all, k_all, v_all]
            for grp in range(4):
                for half in range(2):
                    ps = psA.tile([128, 512], f32, tag="a")
                    ps3 = ps[:].rearrange("p (a b) -> p a b", a=16)
                    for tap in range(9):
                        dy, dx = tap // 3, tap % 3
                        rhs = xp3[:, 16 * half + dy:16 * half + dy + 16,
                                  dx:dx + W]
                        nc.tensor.matmul(ps3, wall_sb[:, grp, tap, :], rhs,
                                         start=(tap == 0), stop=(tap == 8))
                    bias = ball_sb[:, grp:grp + 1]
                    if grp == 0:
                        ocs = outp.tile([128, 512], f32, tag="ocs")
                        nc.scalar.activation(ocs[:], ps[:], AF.Identity,
                                             bias=bias)
                        nc.sync.dma_start(
                            oc_d[:, half * 512:(half + 1) * 512], ocs[:])
                    else:
                        dst = qkv_dst[grp]
                        nc.scalar.activation(
                            dst[:, half * 512:(half + 1) * 512], ps[:],
                            AF.Identity, bias=bias)

            # ---- rel prod: (1024, 504) = q_all^T @ krbd (block diag) ----
            for ic in range(8):
                pp = psA.tile([128, PRODW], f32, tag="a")
                nc.tensor.matmul(pp[:], q_all[:, ic * 128:(ic + 1) * 128],
                                 krbd_sb[:], start=True, stop=True)
                psb = work.tile([128, PRODW], f32r, tag="prodsb")
                nc.vector.tensor_copy(psb[:], pp[:])
                nc.sync.dma_start(prod_d[ic * 128:(ic + 1) * 128, :], psb[:])

            # ---- v^T: transpose all of v (by 128-col chunks) ----
            vt_all = qkvp.tile([128, 8, 128], f32r, tag="vt")
            for jc in range(8):
                tp = psT.tile([128, 128], f32, tag="t")
                nc.tensor.transpose(tp[:], v_all[:, jc * 128:(jc + 1) * 128]
                                    .bitcast(f32), ident[:])
                nc.vector.tensor_copy(vt_all[:, jc, :], tp[:].bitcast(f32r))

            # ---- per-head attention ----
            for h in range(HPC):
                # q_aug = [q_head; RX^T; RY^T]  (96, 1024)
                q_aug = haug.tile([96, HW], f32r, tag="qaug")
                nc.vector.tensor_copy(q_aug[0:32, :],
                                      q_all[32 * h:32 * h + 32, :])
                # k_aug = [k_head; Ex; Ey]  (96, 1024)
                k_aug = haug.tile([96, HW], f32r, tag="kaug")
                nc.vector.tensor_copy(k_aug[0:32, :],
                                      k_all[32 * h:32 * h + 32, :])
                nc.vector.tensor_copy(k_aug[32:64, :], exey_sb[0:32, :])
                nc.vector.tensor_copy(k_aug[64:96, :], exey_sb[32:64, :])

                for ic in range(8):
                    # shear-gather RX, RY for this query chunk from prod_d
                    # prod_d row I=(yI,xI), col h*126 + m (x) / h*126+63+m (y)
                    # RX[p=(yl,xI), c] = prod[I, c+31-xI]
                    # addr = yl*4*PRODW*? ... strides in elements:
                    # I = (ic*4+yl)*32 + xI ; row stride PRODW
                    rx = work.tile([128, 32], f32r, tag="rx")
                    off_x = (ic * 4 * 32) * PRODW + h * RELC + 31
                    src_x = bass.AP(prod_d, off_x,
                                    [[32 * PRODW, 4], [PRODW - 1, 32], [1, 32]])
                    nc.sync.dma_start(rx[:], src_x)
                    # RY[p=(yl,xI), r] = prod[(xI*32 + yI), 63 + r + 31 - xI]
                    #   yI = ic*4 + yl
                    ry = work.tile([128, 32], f32r, tag="ry")
                    off_y = ic * 4 * PRODW + h * RELC + 63 + 31
                    src_y = bass.AP(prod_d, off_y,
                                    [[PRODW, 4], [32 * PRODW - 1, 32], [1, 32]])
                    nc.sync.dma_start(ry[:], src_y)
                    # transpose [RX | RY] (128, 64) -> (64, 128)
                    rxy = work.tile([128, 64], f32r, tag="rxy")
                    nc.vector.tensor_copy(rxy[:, 0:32], rx[:])
                    nc.vector.tensor_copy(rxy[:, 32:64], ry[:])
                    tp = psT.tile([64, 128], f32, tag="t")
                    nc.tensor.transpose(tp[:], rxy[:].bitcast(f32), ident[:])
                    nc.vector.tensor_copy(
                        q_aug[32:64, ic * 128:(ic + 1) * 128],
                        tp[0:32, :].bitcast(f32r))
                    nc.vector.tensor_copy(
                        q_aug[64:96, ic * 128:(ic + 1) * 128],
                        tp[32:64, :].bitcast(f32r))

                # attention: ST = exp(k_aug^T q_aug - 20), AV + denom
                att_ps = psAV.tile([33, HW], f32, tag="attps")
                for jc in range(8):
                    fvt = work.tile([128, 33], f32r, tag="fvt")
                    nc.vector.tensor_copy(fvt[:, 0:32],
                                          vt_all[:, jc, 32 * h:32 * h + 32])
                    nc.vector.tensor_copy(fvt[:, 32:33], ones_r[:])
                    ex = expp.tile([128, HW], f32r, tag="expst")
                    for nh_ in range(2):
                        sl = slice(nh_ * 512, (nh_ + 1) * 512)
                        st = psST.tile([128, 512], f32, tag="st")
                        nc.tensor.matmul(st[:],
                                         k_aug[:, jc * 128:(jc + 1) * 128],
                                         q_aug[:, sl],
                                         start=True, stop=True)
                        nc.scalar.activation(ex[:, sl], st[:], AF.Exp,
                                             bias=negc[:])
                        nc.tensor.matmul(att_ps[:, sl], fvt[:], ex[:, sl],
                                         start=(jc == 0), stop=(jc == 7))

                # normalize + emit att chunks (i on partitions, d free)
                attn_sb = work.tile([33, HW], f32, tag="attnsb")
                nc.vector.tensor_copy(attn_sb[:], att_ps[:])
                for ic in range(8):
                    tp = psT.tile([128, 33], f32, tag="t")
                    nc.tensor.transpose(tp[:],
                                        attn_sb[:, ic * 128:(ic + 1) * 128],
                                        ident[0:33, 0:33])
                    rec = work.tile([128, 1], f32, tag="rec")
                    nc.vector.reciprocal(rec[:], tp[:, 32:33])
                    asb = work.tile([128, 32], f32r, tag="asb")
                    nc.vector.tensor_tensor(asb[:], tp[:, 0:32],
                                            rec[:].to_broadcast((128, 32)),
                                            mybir.AluOpType.mult)
                    # att_d rows h*32 + ic*4 + (0..4), cols = flat (y, x)
                    dst = att_d[h * 32 + ic * 4: h * 32 + ic * 4 + 4, :] \
                        .rearrange("r (a b) -> r a b", a=32)
                    nc.sync.dma_start(dst, asb[:])

            # ---- 1x1 combine conv (partial over this core's 128 channels) ----
            attr_sb = consts.tile([128, HW], f32r)
            nc.sync.dma_start(attr_sb[:], att_d[:])
            for og in range(2):
                for half in range(2):
                    ps = psA.tile([128, 512], f32, tag="a")
                    nc.tensor.matmul(ps[:],
                                     awt_sb[:, og * 128:(og + 1) * 128],
                                     attr_sb[:, half * 512:(half + 1) * 512],
                                     start=True, stop=True)
                    oas = outp.tile([128, 512], f32, tag="oas")
                    nc.scalar.activation(oas[:], ps[:], AF.Identity,
                                         bias=ab_sb[:, og:og + 1])
                    nc.sync.dma_start(
                        oa_d[og * 128:(og + 1) * 128,
                             half * 512:(half + 1) * 512], oas[:])

    nc.finalize()
    return nc


def _prep_inputs(x, conv_w, conv_b, qkv_w, qkv_b, att_w, att_b, kr_x, kr_y):
    """Build the 8 per-core input maps (host-side numpy)."""
    sc = np.float32(DKH ** -0.5)
    # kr block-diagonal: (128, 504)
    krcat = np.concatenate([kr_x.T, kr_y.T], axis=1)        # (32, 126)
    krbd = np.zeros((128, HPC * RELC), np.float32)
    for hh in range(HPC):
        krbd[32 * hh:32 * hh + 32, RELC * hh:RELC * (hh + 1)] = krcat
    # Ex/Ey indicators (64, 1024)
    exey = np.zeros((64, HW), np.float32)
    j = np.arange(HW)
    exey[j % W, j] = 1.0
    exey[32 + j // W, j] = 1.0

    def conv_lhsT(w):                                        # (co,ci,3,3)->(ci,9,co)
        return np.ascontiguousarray(w.transpose(1, 2, 3, 0).reshape(CIN, 9 * 128))

    in_maps = []
    for c in range(NCORES):
        b, g = divmod(c, 2)
        xp = np.zeros((CIN, PADW, PADW), np.float32)
        xp[:, 1:1 + H, 1:1 + W] = x[b]
        s = g * 128
        grps = [
            conv_lhsT(conv_w[s:s + 128]),
            conv_lhsT(qkv_w[s:s + 128] * sc),
            conv_lhsT(qkv_w[DK + s:DK + s + 128]),
            conv_lhsT(qkv_w[2 * DK + s:2 * DK + s + 128]),
        ]
        wall = np.stack(grps, axis=1).reshape(CIN, 4 * 9 * 128)
        ball = np.stack([
            conv_b[s:s + 128],
            qkv_b[s:s + 128] * sc,
            qkv_b[DK + s:DK + s + 128],
            qkv_b[2 * DK + s:2 * DK + s + 128],
        ]).astype(np.float32)
        awt = np.ascontiguousarray(att_w[:, s:s + 128, 0, 0].T)
        ab = (att_b.reshape(2, 128) if g == 0
              else np.zeros((2, 128))).astype(np.float32)
        in_maps.append({
            "xp": np.ascontiguousarray(xp.reshape(CIN, NPIX)),
            "wall": np.ascontiguousarray(wall),
            "ball": ball,
            "krbd": krbd,
            "exey": exey,
            "awt": awt,
            "ab": ab,
        })
    return in_maps


def kernel(x, conv_w, conv_b, qkv_w, qkv_b, att_w, att_b, kr_x, kr_y,
           _trace=False):
    from concourse.bass_utils import run_bass_kernel_spmd

    x = np.asarray(x, np.float32)
    conv_w = np.asarray(conv_w, np.float32)
    conv_b = np.asarray(conv_b, np.float32)
    qkv_w = np.asarray(qkv_w, np.float32)
    qkv_b = np.asarray(qkv_b, np.float32)
    att_w = np.asarray(att_w, np.float32)
    att_b = np.asarray(att_b, np.float32)
    kr_x = np.asarray(kr_x, np.float32)
    kr_y = np.asarray(kr_y, np.float32)

    if "nc" not in _CACHE:
        _CACHE["nc"] = _build_bass()
    nc = _CACHE["nc"]

    in_maps = _prep_inputs(x, conv_w, conv_b, qkv_w, qkv_b, att_w, att_b,
                           kr_x, kr_y)
    res = run_bass_kernel_spmd(nc, in_maps, core_ids=list(range(NCORES)),
                               trace=_trace)
    _CACHE["last_result"] = res

    out = np.empty((B, COUT, H, W), np.float32)
    for b in range(B):
        r0, r1 = res.results[2 * b], res.results[2 * b + 1]
        out[b, 0:128] = r0["oc"].reshape(128, H, W)
        out[b, 128:256] = r1["oc"].reshape(128, H, W)
        out[b, 256:512] = (r0["oa"] + r1["oa"]).reshape(256, H, W)
    return out


# revision 10
# speedup vs baseline: 1.5344x; 1.5344x over previous
"""AttentionAugmentedConv2d Trainium2 kernel (8 NeuronCores, SPMD).

Sharding: core c -> image b = c//2, half g = c%2.
Each core computes, for its image:
  - conv3x3 for 128 of the 256 conv_out channels
  - qkv conv3x3 for its 4 heads (128 q + 128 k + 128 v channels)
  - full attention (with relative position logits) for those 4 heads
  - a partial 1x1 "combine" conv: all 256 output channels contracted over
    its 128 att channels.  Host sums the two halves per image (att bias is
    folded into the g==0 core's bias input).

All matmuls run in float32r (tf32-like, ~2e-4 rel err) at full PE rate.
Relative logits are folded into the QK matmul by augmenting the
contraction dim: logits^T = [k; Ex; Ey]^T @ [q; RX^T; RY^T], with Ex/Ey
constant 0/1 indicator matrices and RX/RY produced by a small q @ kr^T
matmul followed by a shear-gather DMA through DRAM (the rel_to_abs skew
is a linear access pattern in (y, x, c) coordinates).

Logits are computed transposed (keys on partitions, queries free) so that
softmax needs no transpose: exp (constant -20 bias, cancels in
normalization), denominators via a ones-column interleaved into fv^T.
The reference's raw reshape of att (B,NH,HW,dvh)->(B,NH,dvh,H,W) is a
flat memory reinterpretation, obtained for free via a DRAM roundtrip.

Schedule: warmup matmuls keep HAM warm during input DMAs; all shear
gathers are issued in the prologue; heads are software-pipelined (head
h's compute interleaves head h+1's aug-transposes and head h-1's output
normalization) so the PE never sees a low-density window.
"""

import numpy as np

NH = 8
DK = 256
DV = 256
DKH = DK // NH          # 32
DVH = DV // NH          # 32
H = W = 32
HW = H * W              # 1024
B = 4
CIN = 128
COUT = 512
KS = 3
NCORES = 8
HPC = NH // 2           # heads per core = 4
PADW = W + 2            # 34
NPIX = PADW * PADW      # 1156
RELW = 2 * W - 1        # 63
RELC = 2 * RELW         # 126
PRODW = HPC * RELC      # 504

_CACHE = {}


def _build_bass():
    import concourse.bass as bass
    import concourse.mybir as mybir
    from concourse import bacc
    from concourse.tile import TileContext
    from concourse.masks import make_identity

    dt = mybir.dt
    f32 = dt.float32
    f32r = dt.float32r
    AF = mybir.ActivationFunctionType
    MUL = mybir.AluOpType.mult

    nc = bacc.Bacc("TRN2", target_bir_lowering=False, debug=False,
                   num_devices=NCORES)

    # ---- I/O ----
    xp_d = nc.dram_tensor("xp", [CIN, NPIX], f32r, kind="ExternalInput")
    wall_d = nc.dram_tensor("wall", [CIN, 4 * 9 * 128], f32r, kind="ExternalInput")
    ball_d = nc.dram_tensor("ball", [4, 128], f32, kind="ExternalInput")
    krbd_d = nc.dram_tensor("krbd", [128, PRODW], f32r, kind="ExternalInput")
    exey_d = nc.dram_tensor("exey", [64, HW], f32r, kind="ExternalInput")
    awt_d = nc.dram_tensor("awt", [128, 256], f32r, kind="ExternalInput")
    ab_d = nc.dram_tensor("ab", [2, 128], f32, kind="ExternalInput")
    oc_d = nc.dram_tensor("oc", [128, HW], f32, kind="ExternalOutput")
    oa_d = nc.dram_tensor("oa", [256, HW], f32, kind="ExternalOutput")
    prod_d = nc.dram_tensor("prodd", [HW, PRODW], f32r)
    att_d = nc.dram_tensor("attd", [128, HW], f32r)

    with TileContext(nc) as tc:
        with (
            tc.tile_pool(name="consts", bufs=1) as consts,
            tc.tile_pool(name="pers", bufs=1) as pers,
            tc.tile_pool(name="expp", bufs=3) as expp,
            tc.tile_pool(name="work", bufs=3) as work,
            tc.tile_pool(name="outp", bufs=2) as outp,
            tc.tile_pool(name="psA", bufs=4, space="PSUM") as psA,
            tc.tile_pool(name="psAV", bufs=1, space="PSUM") as psAV,
            tc.tile_pool(name="psT", bufs=2, space="PSUM") as psT,
        ):
            # ---------- constants / identities ----------
            ident = consts.tile([128, 128], f32)
            make_identity(nc, ident[:])
            identr = consts.tile([128, 128], f32r)
            nc.vector.tensor_copy(identr[:], ident[:])
            ones_f = consts.tile([128, 1], f32)
            nc.gpsimd.memset(ones_f[:], 1.0)
            ones_r = consts.tile([128, 1], f32r)
            nc.vector.tensor_copy(ones_r[:], ones_f[:])
            negc = consts.tile([128, 1], f32)
            nc.gpsimd.memset(negc[:], -20.0)

            # ---------- input DMAs (ordered by first use) ----------
            xp_sb = consts.tile([CIN, NPIX], f32r)
            nc.sync.dma_start(xp_sb[:], xp_d[:])
            wall_sb = [consts.tile([CIN, 9, 128], f32r, tag=f"wall{g}", name=f"wall{g}")
                       for g in range(4)]
            wd4 = wall_d[:].rearrange("c (g t o) -> c g t o", g=4, t=9)
            for g in (1, 2, 3, 0):   # q, k, v, conv order
                nc.sync.dma_start(wall_sb[g][:], wd4[:, g])
            ball_sb = consts.tile([128, 4], f32)
            nc.sync.dma_start(ball_sb[:], ball_d[:].rearrange("g c -> c g"))
            krbd_sb = consts.tile([128, PRODW], f32r)
            nc.sync.dma_start(krbd_sb[:], krbd_d[:])
            exey_sb = consts.tile([64, HW], f32r)
            nc.scalar.dma_start(exey_sb[:], exey_d[:])
            awt_sb = consts.tile([128, 256], f32r)
            nc.scalar.dma_start(awt_sb[:], awt_d[:])
            ab_sb = consts.tile([128, 2], f32)
            nc.scalar.dma_start(ab_sb[:], ab_d[:].rearrange("g c -> c g"))

            # ---------- HAM warmup (runs while DMAs land) ----------
            warm_f = consts.tile([128, 512], f32)
            nc.gpsimd.memset(warm_f[:], 0.5)
            warm_r = consts.tile([128, 512], f32r)
            nc.vector.tensor_copy(warm_r[:], warm_f[:])
            for wi in range(10):
                wps = psA.tile([128, 512], f32, tag="a")
                nc.tensor.matmul(wps[:], warm_r[:, 0:128], warm_r[:],
                                 start=True, stop=True)

            # ---------- persistent tiles ----------
            q_all = pers.tile([128, HW], f32r, tag="q")
            k_all = pers.tile([128, HW], f32r, tag="k")
            v_all = pers.tile([128, HW], f32r, tag="v")
            vt_all = pers.tile([128, 8, 34 * HPC], f32r, tag="vt")
            qaug = [pers.tile([96, HW], f32r, tag=f"qaug{h}", name=f"qaug{h}") for h in range(HPC)]
            kaug = [pers.tile([96, HW], f32r, tag=f"kaug{h}", name=f"kaug{h}") for h in range(HPC)]
            rxa = [pers.tile([128, 8, 32], f32r, tag=f"rxa{h}", name=f"rxa{h}") for h in range(HPC)]
            rya = [pers.tile([128, 8, 32], f32r, tag=f"rya{h}", name=f"rya{h}") for h in range(HPC)]
            attn_sb = [pers.tile([33, HW], f32r, tag=f"attn{h}", name=f"attn{h}")
                       for h in range(2)]
            att_hd = [pers.tile([128, 8, 32], f32r, tag=f"ahd{h}", name=f"ahd{h}")
                      for h in range(2)]
            attr_sb = pers.tile([128, HW], f32r, tag="attr")

            # ones columns interleaved into vt_all (col 32 of each 34-block)
            vt4 = vt_all[:].rearrange("p a (h c) -> p a h c", h=HPC)
            nc.vector.tensor_copy(
                vt4[:, :, :, 32:33],
                ones_r[:].unsqueeze(1).unsqueeze(1).to_broadcast((128, 8, HPC, 1)))

            # ---------- convs ----------
            xp3 = xp_sb[:].rearrange("c (a b) -> c a b", a=PADW)

            def conv_group(grp, dst):
                for half in range(2):
                    ps = psA.tile([128, 512], f32, tag="a")
                    ps3 = ps[:].rearrange("p (a b) -> p a b", a=16)
                    for tap in range(9):
                        dy, dx = tap // 3, tap % 3
                        rhs = xp3[:, 16 * half + dy:16 * half + dy + 16,
                                  dx:dx + W]
                        nc.tensor.matmul(ps3, wall_sb[grp][:, tap, :], rhs,
                                         start=(tap == 0), stop=(tap == 8))
                    bias = ball_sb[:, grp:grp + 1]
                    if dst is None:
                        ocs = outp.tile([128, 512], f32, tag="ocs")
                        nc.scalar.activation(ocs[:], ps[:], AF.Identity,
                                             bias=bias)
                        nc.sync.dma_start(
                            oc_d[:, half * 512:(half + 1) * 512], ocs[:])
                    else:
                        nc.scalar.activation(
                            dst[:, half * 512:(half + 1) * 512], ps[:],
                            AF.Identity, bias=bias)

            conv_group(1, q_all)

            # rel prod (depends only on q)
            for ic in range(8):
                pp = psA.tile([128, PRODW], f32, tag="a")
                nc.tensor.matmul(pp[:], q_all[:, ic * 128:(ic + 1) * 128],
                                 krbd_sb[:], start=True, stop=True)
                psb = work.tile([128, PRODW], f32r, tag="prodsb")
                nc.vector.tensor_copy(psb[:], pp[:])
                nc.gpsimd.dma_start(prod_d[ic * 128:(ic + 1) * 128, :], psb[:])

            conv_group(2, k_all)
            conv_group(3, v_all)

            # v^T with per-head ones columns
            for jc in range(8):
                tp = psT.tile([128, 128], f32, tag="t")
                nc.tensor.transpose(tp[:], v_all[:, jc * 128:(jc + 1) * 128]
                                    .bitcast(f32), ident[:])
                for hh in range(HPC):
                    nc.vector.tensor_copy(
                        vt_all[:, jc, 34 * hh:34 * hh + 32],
                        tp[:, 32 * hh:32 * hh + 32])

            conv_group(0, None)

            # ---------- prologue aug DMAs (all heads) ----------
            def shear_dmas(h):
                eng = nc.sync if h % 2 == 0 else nc.scalar
                for ic in range(8):
                    # RX[(yl,xI), c] = prod[(4ic+yl)*32+xI, h*126 + c+31-xI]
                    off_x = h * RELC + 31 + ic * 128 * PRODW
                    src_x = bass.AP(prod_d, off_x,
                                    [[32 * PRODW, 4], [PRODW - 1, 32], [1, 32]])
                    eng.dma_start(rxa[h][:, ic, :], src_x)
                    # RY[(yl,xI), r] = prod[xI*32+4ic+yl, h*126+63 + r+31-xI]
                    off_y = h * RELC + 63 + 31 + ic * 4 * PRODW
                    src_y = bass.AP(prod_d, off_y,
                                    [[PRODW, 4], [32 * PRODW - 1, 32], [1, 32]])
                    eng.dma_start(rya[h][:, ic, :], src_y)

            for h in range(HPC):
                shear_dmas(h)
                # q/k head rows + exey into per-head aug tiles (SBUF->SBUF DMA)
                nc.sync.dma_start(qaug[h][0:32, :], q_all[32 * h:32 * h + 32, :])
                nc.sync.dma_start(kaug[h][0:32, :], k_all[32 * h:32 * h + 32, :])
                nc.scalar.dma_start(kaug[h][32:96, :], exey_sb[:])

            # aug transposes for head h, chunk-pair p (ics 2p, 2p+1)
            def aug_T(h, p, kind):
                src = (rxa if kind == 0 else rya)[h]
                rowbase = 32 + 32 * kind
                tp = psT.tile([128, 128], f32, tag="t")
                nc.tensor.transpose(tp[0:64, :],
                                    src[:, 2 * p:2 * p + 2, :].bitcast(f32),
                                    ident[:])
                for k2 in range(2):
                    ic = 2 * p + k2
                    nc.vector.tensor_copy(
                        qaug[h][rowbase:rowbase + 32,
                                ic * 128:(ic + 1) * 128],
                        tp[32 * k2:32 * k2 + 32, :])

            for p in range(4):
                aug_T(0, p, 0)
                aug_T(0, p, 1)

            # ---------- head pipeline ----------
            def emit_ST(h, jc):
                ex = expp.tile([128, HW], f32r, tag="ex")
                for nh_ in range(2):
                    sl = slice(nh_ * 512, (nh_ + 1) * 512)
                    st = psA.tile([128, 512], f32, tag="a")
                    nc.tensor.matmul(st[:],
                                     kaug[h][:, jc * 128:(jc + 1) * 128],
                                     qaug[h][:, sl], start=True, stop=True)
                    nc.scalar.activation(ex[:, sl], st[:], AF.Exp,
                                         bias=negc[:])
                return ex

            def emit_AV(h, jc, ex, att_ps):
                for nh_ in range(2):
                    sl = slice(nh_ * 512, (nh_ + 1) * 512)
                    nc.tensor.matmul(att_ps[:, sl],
                                     vt_all[:, jc, 34 * h:34 * h + 33],
                                     ex[:, sl],
                                     start=(jc == 0), stop=(jc == 7))

            def fin_ic(h, ic):
                asb = attn_sb[h % 2]
                tp = psT.tile([128, 128], f32, tag="t")
                nc.tensor.transpose(tp[:, 0:33],
                                    asb[:, ic * 128:(ic + 1) * 128]
                                    .bitcast(f32), ident[0:33, 0:33])
                rec = work.tile([128, 1], f32, tag="rec")
                nc.vector.reciprocal(rec[:], tp[:, 32:33])
                nc.vector.tensor_tensor(att_hd[h % 2][:, ic, :],
                                        tp[:, 0:32],
                                        rec[:].to_broadcast((128, 32)), MUL)

            def fin_wr(h):
                for ic in range(8):
                    dst = bass.AP(att_d, (32 * h + 4 * ic) * HW,
                                  [[HW, 4], [32, 32], [1, 32]])
                    nc.gpsimd.dma_start(dst, att_hd[h % 2][:, ic, :])

            def fin_rd(h):
                nc.sync.dma_start(attr_sb[32 * h:32 * h + 32, :],
                                  att_d[32 * h:32 * h + 32, :])

            for h in range(HPC):
                att_ps = psAV.tile([33, HW], f32, tag="av")
                exs = {}
                for jc in range(8):
                    exs[jc] = emit_ST(h, jc)
                    if jc >= 1:
                        emit_AV(h, jc - 1, exs.pop(jc - 1), att_ps)
                    if h + 1 < HPC and jc >= 4:
                        aug_T(h + 1, jc - 4, 0)
                        aug_T(h + 1, jc - 4, 1)
                    if h > 0:
                        fin_ic(h - 1, jc)
                emit_AV(h, 7, exs.pop(7), att_ps)
                if h > 0:
                    fin_wr(h - 1)
                    fin_rd(h - 1)
                # att_ps -> sbuf (two halves to shrink the copy latency)
                nc.vector.tensor_copy(attn_sb[h % 2][:, 0:512],
                                      att_ps[:, 0:512])
                nc.vector.tensor_copy(attn_sb[h % 2][:, 512:HW],
                                      att_ps[:, 512:HW])

            for ic in range(8):
                fin_ic(3, ic)
            fin_wr(3)
            fin_rd(3)

            # ---------- 1x1 combine conv ----------
            for og in range(2):
                for half in range(2):
                    ps = psA.tile([128, 512], f32, tag="a")
                    nc.tensor.matmul(ps[:],
                                     awt_sb[:, og * 128:(og + 1) * 128],
                                     attr_sb[:, half * 512:(half + 1) * 512],
                                     start=True, stop=True)
                    oas = outp.tile([128, 512], f32, tag="oas")
                    nc.scalar.activation(oas[:], ps[:], AF.Identity,
                                         bias=ab_sb[:, og:og + 1])
                    nc.sync.dma_start(
                        oa_d[og * 128:(og + 1) * 128,
                             half * 512:(half + 1) * 512], oas[:])

    nc.finalize()
    return nc


def _prep_inputs(x, conv_w, conv_b, qkv_w, qkv_b, att_w, att_b, kr_x, kr_y):
    """Build the 8 per-core input maps (host-side numpy)."""
    sc = np.float32(DKH ** -0.5)
    krcat = np.concatenate([kr_x.T, kr_y.T], axis=1)        # (32, 126)
    krbd = np.zeros((128, PRODW), np.float32)
    for hh in range(HPC):
        krbd[32 * hh:32 * hh + 32, RELC * hh:RELC * (hh + 1)] = krcat
    exey = np.zeros((64, HW), np.float32)
    j = np.arange(HW)
    exey[j % W, j] = 1.0
    exey[32 + j // W, j] = 1.0

    def conv_lhsT(w):                                        # (co,ci,3,3)->(ci,9,co)
        return np.ascontiguousarray(w.transpose(1, 2, 3, 0).reshape(CIN, 9 * 128))

    in_maps = []
    for c in range(NCORES):
        b, g = divmod(c, 2)
        xp = np.zeros((CIN, PADW, PADW), np.float32)
        xp[:, 1:1 + H, 1:1 + W] = x[b]
        s = g * 128
        grps = [
            conv_lhsT(conv_w[s:s + 128]),
            conv_lhsT(qkv_w[s:s + 128] * sc),
            conv_lhsT(qkv_w[DK + s:DK + s + 128]),
            conv_lhsT(qkv_w[2 * DK + s:2 * DK + s + 128]),
        ]
        wall = np.stack(grps, axis=1).reshape(CIN, 4 * 9 * 128)
        ball = np.stack([
            conv_b[s:s + 128],
            qkv_b[s:s + 128] * sc,
            qkv_b[DK + s:DK + s + 128],
            qkv_b[2 * DK + s:2 * DK + s + 128],
        ]).astype(np.float32)
        awt = np.ascontiguousarray(att_w[:, s:s + 128, 0, 0].T)
        ab = (att_b.reshape(2, 128) if g == 0
              else np.zeros((2, 128))).astype(np.float32)
        in_maps.append({
            "xp": np.ascontiguousarray(xp.reshape(CIN, NPIX)),
            "wall": np.ascontiguousarray(wall),
            "ball": ball,
            "krbd": krbd,
            "exey": exey,
            "awt": awt,
            "ab": ab,
        })
    return in_maps


def kernel(x, conv_w, conv_b, qkv_w, qkv_b, att_w, att_b, kr_x, kr_y,
           _trace=False):
    from concourse.bass_utils import run_bass_kernel_spmd

    x = np.asarray(x, np.float32)
    conv_w = np.asarray(conv_w, np.float32)
    conv_b = np.asarray(conv_b, np.float32)
    qkv_w = np.asarray(qkv_w, np.float32)
    qkv_b = np.asarray(qkv_b, np.float32)
    att_w = np.asarray(att_w, np.float32)
    att_b = np.asarray(att_b, np.float32)
    kr_x = np.asarray(kr_x, np.float32)
    kr_y = np.asarray(kr_y, np.float32)

    if "nc" not in _CACHE:
        _CACHE["nc"] = _build_bass()
    nc = _CACHE["nc"]

    in_maps = _prep_inputs(x, conv_w, conv_b, qkv_w, qkv_b, att_w, att_b,
                           kr_x, kr_y)
    res = run_bass_kernel_spmd(nc, in_maps, core_ids=list(range(NCORES)),
                               trace=_trace)
    _CACHE["last_result"] = res

    out = np.empty((B, COUT, H, W), np.float32)
    for b in range(B):
        r0, r1 = res.results[2 * b], res.results[2 * b + 1]
        out[b, 0:128] = r0["oc"].reshape(128, H, W)
        out[b, 128:256] = r1["oc"].reshape(128, H, W)
        out[b, 256:512] = (r0["oa"] + r1["oa"]).reshape(256, H, W)
    return out
